# revision 1
# baseline (speedup 1.0000x reference)
"""GumbelSparseAttention kernel for 8 Trainium2 NeuronCores.

Reference semantics (B=1, L=2048, E=1024, H=16, d=64, TAU=0.1):
  scores = (q @ k^T) * d**-0.5                     per head   [L, L]
  logits = q.mean(-1) @ w_gumbel^T + b_gumbel      per head   [L]
  mask   = one_hot(argmax(logits + gumbel(u)))  (+ y - y = fp-exact one_hot)
  out[l] = softmax(scores[l] * mask[l]) @ v
Because mask is a one-hot over the *query* axis, only one row per head gets
real attention; every other row's scores are exactly 0 -> uniform softmax ->
out row = mean(v).  The kernel computes per head: the logits argmax, one
attention row, and the v column means.

Sharding (8 cores): w_gumbel split by columns (contraction j) -> partial
logits [16, L] per core -> ReduceScatter(add) gives each core the summed
logits for its own 2 heads.  k/v/heads split 2-per-core.  No other comm.
"""

import sys

sys.path.insert(0, "/opt/trn_rl_repo")

import numpy as np  # noqa: E402
import concourse.bass as bass  # noqa: E402
import concourse.mybir as mybir  # noqa: E402
import concourse.tile as tile  # noqa: E402
from concourse.tile import TileContext  # noqa: E402
from concourse.masks import make_identity  # noqa: E402
from concourse.vector_clock import ScopedClock, VectorClock  # noqa: E402

F32 = mybir.dt.float32
I32 = mybir.dt.int32
U32 = mybir.dt.uint32

N_CORES = 8
L = 2048
E = 1024
H = 16
D = 64
HPC = H // N_CORES          # heads per core = 2
JC = L // N_CORES           # w_gumbel column chunk = 256
QC = L // N_CORES           # q row chunk = 256
SCALE = D ** -0.5           # 0.125
AF = mybir.ActivationFunctionType
ALU = mybir.AluOpType


# ---------------------------------------------------------------------------
# Workarounds for this toolchain's walrus: it rejects instructions carrying
# more than ~2 semaphore waits, including the Tile tail drain.
# ---------------------------------------------------------------------------

def _patched_drain_and_barrier(self, tick_clock, wait_clock):
    gc = tick_clock.global_clock
    n = len(gc)
    for i in range(n):
        t = gc[i]
        if t > 0:
            vec = [0] * n
            vec[i] = t
            nop = self.nc.sync.nop()
            wait_clock.add_sem_waits(nop.ins, ScopedClock({None: VectorClock(vec)}))
    self.nc.sync.drain()  # waits already handled by the NOP cascade above
    self.nc.all_engine_barrier()
    assert self.sems is not None
    popped = self.nc._tile_sem_poison_stack.pop()
    assert popped is self._sem_poison
    self.nc.clear_and_free_semaphores(list(self.sems.allocated().values()))
    self.nc.all_engine_barrier()


tile.TileContext._drain_and_barrier = _patched_drain_and_barrier


def _split_excess_waits(nc, max_waits=1):
    nsplit = 0
    for fn in nc.m.functions:
        for blk in fn.blocks:
            insts = list(blk.instructions)
            new = []
            for ins in insts:
                si = ins.sync_info
                if si is not None and len(si.on_wait) > max_waits:
                    waits = list(si.on_wait)
                    keep = waits[-max_waits:]
                    for k, w in enumerate(waits[:-max_waits]):
                        nop = mybir.InstNoOp(name=f"{ins.name}-wsplit{k}")
                        nop.engine = ins.engine
                        nop.sync_info = mybir.SyncInfo(on_wait=[w], on_update=[])
                        new.append(nop)
                        nsplit += 1
                    si.on_wait = keep
                new.append(ins)
            blk.instructions = new
    return nsplit


# ---------------------------------------------------------------------------
# Device program
# ---------------------------------------------------------------------------

_CACHE = {}

_MASK2 = np.zeros((HPC, HPC * D), np.float32)
for _h in range(HPC):
    _MASK2[_h, _h * D:(_h + 1) * D] = 1.0


def _build_program():
    nc = bass.Bass("TRN2", num_devices=N_CORES)

    qchunk = nc.dram_tensor("qchunk", [QC, E], F32, kind="ExternalInput")
    wchunk = nc.dram_tensor("wchunk", [L, JC], F32, kind="ExternalInput")
    kh = nc.dram_tensor("kh", [L, HPC * D], F32, kind="ExternalInput")
    vh = nc.dram_tensor("vh", [L, HPC * D], F32, kind="ExternalInput")
    qfull = nc.dram_tensor("qfull", [L * H, D], F32, kind="ExternalInput")
    upair = nc.dram_tensor("upair", [HPC, L], F32, kind="ExternalInput")
    bpair = nc.dram_tensor("bpair", [HPC, L], F32, kind="ExternalInput")
    hoff = nc.dram_tensor("hoff", [HPC, 1], I32, kind="ExternalInput")
    maskin = nc.dram_tensor("maskin", [HPC, HPC * D], F32, kind="ExternalInput")
    outd = nc.dram_tensor("out", [L, HPC * D], F32, kind="ExternalOutput")

    lpart = nc.dram_tensor("lpart", [H, L], F32)
    lrs = nc.dram_tensor("lrs", [HPC, L], F32)

    NCH = L // 128  # 16 row chunks

    with TileContext(nc) as tc:
        # PSUM budget is 8 banks total (2KB/partition each), statically
        # reserved per pool*tag*bufs: ps_tr 2 + ps_mm 2 + ps_acc 2 + ps_sm 2.
        with tc.tile_pool(name="big", bufs=1) as big, \
             tc.tile_pool(name="work", bufs=1) as work, \
             tc.tile_pool(name="ps_tr", bufs=2, space="PSUM") as ps_tr, \
             tc.tile_pool(name="ps_mm", bufs=2, space="PSUM") as ps_mm, \
             tc.tile_pool(name="ps_acc", bufs=1, space="PSUM") as ps_acc, \
             tc.tile_pool(name="ps_sm", bufs=2, space="PSUM") as ps_sm:

            ident = work.tile([128, 128], F32)
            make_identity(nc, ident)

            # ---- load w chunk and transpose to [j, i] layout ----------------
            wnat = big.tile([128, 16 * JC], F32, tag="wnat")
            nc.sync.dma_start(
                out=wnat[:].rearrange("p (r j) -> p r j", j=JC),
                in_=wchunk.rearrange("(r p) j -> p r j", p=128),
            )
            wT = [big.tile([128, L], F32, tag=f"wT{s}", name=f"wT{s}") for s in range(2)]
            for s in range(2):
                for g in range(4):  # groups of 4 transposes -> one [128,512] copy
                    pt = ps_tr.tile([128, 512], F32, tag="tr")
                    for t in range(4):
                        r = g * 4 + t
                        nc.tensor.transpose(
                            out=pt[:, t * 128:(t + 1) * 128],
                            in_=wnat[:, r * JC + s * 128: r * JC + (s + 1) * 128],
                            identity=ident[:],
                        )
                    nc.vector.tensor_copy(wT[s][:, g * 512:(g + 1) * 512], pt[:])

            # ---- q_mean^T for this j-chunk: [128, 16] x2 --------------------
            qmT = []
            for s in range(2):
                qt = big.tile([128, E], F32, tag=f"qrows{s}")
                nc.sync.dma_start(out=qt[:], in_=qchunk[s * 128:(s + 1) * 128, :])
                qm = work.tile([128, H], F32, tag=f"qmT{s}")
                nc.vector.reduce_sum(
                    qm[:], qt[:].rearrange("p (h d) -> p h d", d=D),
                    axis=mybir.AxisListType.X,
                )
                nc.vector.tensor_scalar_mul(qm[:], qm[:], 1.0 / D)
                qmT.append(qm)

            # ---- partial logits [16, L] on PE, then ReduceScatter -----------
            lp = work.tile([H, L], F32, tag="lp")
            for n in range(4):
                pl = ps_mm.tile([H, 512], F32, tag="mm")
                for s in range(2):
                    nc.tensor.matmul(
                        out=pl[:],
                        lhsT=qmT[s][:],
                        rhs=wT[s][:, n * 512:(n + 1) * 512],
                        start=(s == 0), stop=(s == 1),
                    )
                nc.vector.tensor_copy(lp[:, n * 512:(n + 1) * 512], pl[:])
            nc.sync.dma_start(out=lpart[:], in_=lp[:])
            nc.gpsimd.collective_compute(
                "ReduceScatter", ALU.add,
                replica_groups=[list(range(N_CORES))],
                ins=[lpart[:]], outs=[lrs[:]],
            )

            # ---- k/v load + K transpose (overlaps the collective) -----------
            kt = big.tile([128, NCH * 128], F32, tag="kt")
            nc.sync.dma_start(
                out=kt[:].rearrange("p (r c) -> p r c", c=HPC * D),
                in_=kh.rearrange("(r p) c -> p r c", p=128),
            )
            vt = big.tile([128, NCH * 128], F32, tag="vt")
            nc.sync.dma_start(
                out=vt[:].rearrange("p (r c) -> p r c", c=HPC * D),
                in_=vh.rearrange("(r p) c -> p r c", p=128),
            )
            KT = [big.tile([64, L], F32, tag=f"KT{s}", name=f"KT{s}") for s in range(2)]
            for s in range(2):
                for g in range(4):
                    pk = ps_tr.tile([64, 512], F32, tag="tr")
                    for t in range(4):
                        r = g * 4 + t
                        nc.tensor.transpose(
                            out=pk[:, t * 128:(t + 1) * 128],
                            in_=kt[:, r * 128 + s * 64: r * 128 + (s + 1) * 64],
                            identity=ident[:],
                        )
                    nc.scalar.copy(KT[s][:, g * 512:(g + 1) * 512], pk[:])

            # ---- keep PE in high-activity mode across the collective --------
            for wrm in range(28):
                pw = ps_tr.tile([128, 512], F32, tag="tr", name=f"warm{wrm}")
                nc.tensor.transpose(out=pw[:, 0:128], in_=kt[:, 0:128], identity=ident[:])

            # ---- gumbel + bias + summed logits -> argmax per head -----------
            ut = work.tile([HPC, L], F32, tag="ut")
            nc.sync.dma_start(out=ut[:], in_=upair[:])
            bt = work.tile([HPC, L], F32, tag="bt")
            nc.sync.dma_start(out=bt[:], in_=bpair[:])
            hof = work.tile([HPC, 1], I32, tag="hof")
            nc.sync.dma_start(out=hof[:], in_=hoff[:])

            s1 = work.tile([HPC, L], F32, tag="s1")
            nc.scalar.activation(s1[:], ut[:], AF.Ln)
            s2 = work.tile([HPC, L], F32, tag="s2")
            nc.scalar.activation(s2[:], s1[:], AF.Ln, scale=-1.0)

            bs2 = work.tile([HPC, L], F32, tag="bs2")
            nc.vector.tensor_tensor(out=bs2[:], in0=bt[:], in1=s2[:], op=ALU.subtract)
            lr = work.tile([HPC, L], F32, tag="lr")
            nc.sync.dma_start(out=lr[:], in_=lrs[:])
            z = work.tile([HPC, L], F32, tag="z")
            nc.vector.tensor_tensor(out=z[:], in0=lr[:], in1=bs2[:], op=ALU.add)

            mx = work.tile([HPC, 8], F32, tag="mx")
            idx = work.tile([HPC, 8], U32, tag="idx")
            nc.vector.max_with_indices(mx[:], idx[:], z[:])
            idx_i = work.tile([HPC, 1], I32, tag="idx_i")
            nc.vector.tensor_copy(idx_i[:], idx[:, 0:1])

            # ---- gather the two selected q rows -----------------------------
            fi = work.tile([HPC, 1], I32, tag="fi")
            nc.vector.tensor_scalar(out=fi[:], in0=idx_i[:], scalar1=H,
                                    scalar2=None, op0=ALU.mult)
            nc.vector.tensor_tensor(out=fi[:], in0=fi[:], in1=hof[:], op=ALU.add)
            qsel = work.tile([HPC, D], F32, tag="qsel")
            nc.gpsimd.indirect_dma_start(
                out=qsel[:], out_offset=None,
                in_=qfull[:, :],
                in_offset=bass.IndirectOffsetOnAxis(ap=fi[:, 0:1], axis=0),
            )
            nc.vector.tensor_scalar_mul(qsel[:], qsel[:], SCALE)
            pq = ps_sm.tile([64, HPC], F32, tag="sm")
            nc.tensor.transpose(out=pq[:], in_=qsel[:], identity=ident[0:HPC, 0:HPC])
            qbd = []
            for h in range(2):
                qb = work.tile([64, HPC], F32, tag=f"qbd{h}")
                nc.vector.memset(qb[:], 0.0)
                nc.vector.tensor_copy(qb[:, h:h + 1], pq[:, h:h + 1])
                qbd.append(qb)

            # ---- one attention row per head ---------------------------------
            scsb = work.tile([HPC, L], F32, tag="scsb")
            for n in range(4):
                psc = ps_mm.tile([HPC, 512], F32, tag="mm")
                nc.tensor.matmul(out=psc[:], lhsT=qbd[0][:],
                                 rhs=KT[0][:, n * 512:(n + 1) * 512],
                                 start=True, stop=False)
                nc.tensor.matmul(out=psc[:], lhsT=qbd[1][:],
                                 rhs=KT[1][:, n * 512:(n + 1) * 512],
                                 start=False, stop=True)
                nc.vector.tensor_copy(scsb[:, n * 512:(n + 1) * 512], psc[:])
            smax = work.tile([HPC, 8], F32, tag="smax")
            nc.vector.max(smax[:], scsb[:])
            nmx = work.tile([HPC, 1], F32, tag="nmx")
            nc.vector.tensor_scalar_mul(nmx[:], smax[:, 0:1], -1.0)
            esc = work.tile([HPC, L], F32, tag="esc")
            ssum = work.tile([HPC, 1], F32, tag="ssum")
            nc.scalar.activation(esc[:], scsb[:], AF.Exp, bias=nmx[:], scale=1.0,
                                 accum_out=ssum[:])
            rsum = work.tile([HPC, 1], F32, tag="rsum")
            nc.vector.reciprocal(rsum[:], ssum[:])

            # escores^T into [128, 3] blocks (col 3c+2 stays 1.0 for v colsums)
            escT = work.tile([128, 3 * NCH], F32, tag="escT")
            nc.vector.memset(escT[:], 1.0)
            for g in range(4):
                pe = ps_tr.tile([128, 4 * HPC], F32, tag="tr")
                for t in range(4):
                    r = g * 4 + t
                    nc.tensor.transpose(
                        out=pe[:, t * HPC:(t + 1) * HPC],
                        in_=esc[:, r * 128:(r + 1) * 128],
                        identity=ident[0:HPC, 0:HPC],
                    )
                for t in range(4):
                    r = g * 4 + t
                    nc.vector.tensor_copy(
                        escT[:, 3 * r:3 * r + 2], pe[:, t * HPC:(t + 1) * HPC]
                    )

            # ---- attn row + v column sums (accumulate over 16 chunks) -------
            patt = ps_acc.tile([HPC, 128], F32, tag="patt")
            pvm = ps_acc.tile([1, 128], F32, tag="pvm")
            for r in range(NCH):
                nc.tensor.matmul(out=patt[:], lhsT=escT[:, 3 * r:3 * r + 2],
                                 rhs=vt[:, r * 128:(r + 1) * 128],
                                 start=(r == 0), stop=(r == NCH - 1))
            for r in range(NCH):
                nc.tensor.matmul(out=pvm[:], lhsT=escT[:, 3 * r + 2:3 * r + 3],
                                 rhs=vt[:, r * 128:(r + 1) * 128],
                                 start=(r == 0), stop=(r == NCH - 1))

            vm0 = work.tile([1, 128], F32, tag="vm0")
            nc.vector.tensor_scalar_mul(vm0[:], pvm[:], 1.0 / L)
            att = work.tile([HPC, 128], F32, tag="att")
            nc.vector.tensor_scalar_mul(att[:], patt[:], rsum[:, 0:1])

            ones12 = work.tile([1, HPC], F32, tag="ones12")
            nc.vector.memset(ones12[:], 1.0)
            pvm2 = ps_sm.tile([HPC, 128], F32, tag="sm")
            nc.tensor.matmul(out=pvm2[:], lhsT=ones12[:], rhs=vm0[:],
                             start=True, stop=True)
            mask2 = work.tile([HPC, 128], F32, tag="mask2")
            nc.sync.dma_start(out=mask2[:], in_=maskin[:])
            delta = work.tile([HPC, 128], F32, tag="delta")
            nc.vector.tensor_tensor(out=delta[:], in0=att[:], in1=pvm2[:],
                                    op=ALU.subtract)
            nc.vector.tensor_tensor(out=delta[:], in0=delta[:], in1=mask2[:],
                                    op=ALU.mult)

            # ---- one-hot rows and the output chunks -------------------------
            iot = work.tile([HPC, L], I32, tag="iot")
            nc.gpsimd.iota(iot[:], pattern=[[1, L]], base=0, channel_multiplier=0)
            ohT = work.tile([HPC, L], F32, tag="ohT")
            nc.vector.tensor_tensor(out=ohT[:], in0=iot[:],
                                    in1=idx_i[:].to_broadcast([HPC, L]),
                                    op=ALU.is_equal)
            ones_row = work.tile([1, 128], F32, tag="ones_row")
            nc.vector.memset(ones_row[:], 1.0)
            pvb = ps_sm.tile([128, 128], F32, tag="sm")
            nc.tensor.matmul(out=pvb[:], lhsT=ones_row[:], rhs=vm0[:],
                             start=True, stop=True)
            vmb = work.tile([128, 128], F32, tag="vmb")
            nc.vector.tensor_copy(vmb[:], pvb[:])

            for r in range(NCH):
                po = ps_sm.tile([128, 128], F32, tag="sm")
                nc.tensor.matmul(out=po[:], lhsT=ohT[:, r * 128:(r + 1) * 128],
                                 rhs=delta[:], start=True, stop=True)
                so = work.tile([128, 128], F32, tag=f"so{r % 4}")
                nc.vector.tensor_tensor(out=so[:], in0=po[:], in1=vmb[:], op=ALU.add)
                nc.sync.dma_start(out=outd[r * 128:(r + 1) * 128, :], in_=so[:])

    _split_excess_waits(nc)
    return nc


def kernel(query, key, value, w_gumbel, b_gumbel, gumbel_u):
    from concourse.bass_utils import run_bass_kernel_spmd

    if "nc" not in _CACHE:
        _CACHE["nc"] = _build_program()
    nc = _CACHE["nc"]

    query = np.ascontiguousarray(query, dtype=np.float32)
    key = np.ascontiguousarray(key, dtype=np.float32)
    value = np.ascontiguousarray(value, dtype=np.float32)
    w_gumbel = np.ascontiguousarray(w_gumbel, dtype=np.float32)
    b_gumbel = np.ascontiguousarray(b_gumbel, dtype=np.float32)
    gumbel_u = np.ascontiguousarray(gumbel_u, dtype=np.float32)

    q2 = query.reshape(L, E)
    k2 = key.reshape(L, E)
    v2 = value.reshape(L, E)
    qfull = query.reshape(L * H, D)
    bpair = np.ascontiguousarray(np.broadcast_to(b_gumbel[None, :], (HPC, L)))

    in_maps = []
    for c in range(N_CORES):
        cols = slice(c * HPC * D, (c + 1) * HPC * D)
        in_maps.append({
            "qchunk": np.ascontiguousarray(q2[c * QC:(c + 1) * QC, :]),
            "wchunk": np.ascontiguousarray(w_gumbel[:, c * JC:(c + 1) * JC]),
            "kh": np.ascontiguousarray(k2[:, cols]),
            "vh": np.ascontiguousarray(v2[:, cols]),
            "qfull": qfull,
            "upair": np.ascontiguousarray(gumbel_u[0, c * HPC:(c + 1) * HPC, :]),
            "bpair": bpair,
            "hoff": np.array([[c * HPC], [c * HPC + 1]], dtype=np.int32),
            "maskin": _MASK2,
        })

    res = run_bass_kernel_spmd(nc, in_maps, core_ids=list(range(N_CORES)))
    out = np.concatenate([res.results[c]["out"] for c in range(N_CORES)], axis=1)
    return out.reshape(1, L, E)


if __name__ == "__main__":
    rng = np.random.default_rng(0)
    ins = {
        "query": rng.standard_normal((1, L, E)).astype(np.float32),
        "key": rng.standard_normal((1, L, E)).astype(np.float32),
        "value": rng.standard_normal((1, L, E)).astype(np.float32),
        "w_gumbel": (rng.standard_normal((L, L)) * 0.02).astype(np.float32),
        "b_gumbel": np.zeros(L, np.float32),
        "gumbel_u": rng.uniform(1e-6, 1 - 1e-6, (1, H, L)).astype(np.float32),
    }
    out = kernel(**ins)
    print("out", out.shape, out.dtype, np.abs(out).max())



# revision 3
# speedup vs baseline: 1.2082x; 1.2082x over previous
"""GumbelSparseAttention kernel for 8 Trainium2 NeuronCores.

Reference semantics (B=1, L=2048, E=1024, H=16, d=64, TAU=0.1):
  scores = (q @ k^T) * d**-0.5                     per head   [L, L]
  logits = q.mean(-1) @ w_gumbel^T + b_gumbel      per head   [L]
  mask   = one_hot(argmax(logits + gumbel(u)))  (+ y - y = fp-exact one_hot)
  out[l] = softmax(scores[l] * mask[l]) @ v
The mask is a one-hot over the *query* axis: only one row per head gets real
attention; every other row's scores are exactly 0 -> uniform softmax ->
out row = mean(v).  Per head the kernel computes: the logits argmax, one
attention row, and the v column means.

Sharding (8 cores): w_gumbel split by columns (contraction j) -> partial
logits [16, L] per core -> ReduceScatter(add) gives each core the summed
logits (with bias+gumbel/8 pre-folded) for its own 2 heads.  k/v/heads
split 2-per-core.  w^T and k^T are pre-transposed on the host so the PE
does no layout transposes; all matmuls run with bf16 inputs (fp32 PSUM
accumulation; verified rel-err ~2.5e-3 and argmax-exact for the reference
input distribution).  The RS and the argmax run in fp32 (min top-2 gumbel
gap ~0.011 is below bf16 resolution).  vmean rows are written to the
output during the collective; the tail only fixes up 2 rows per core via
one indirect DMA.
"""

import sys

sys.path.insert(0, "/opt/trn_rl_repo")

import numpy as np  # noqa: E402
import ml_dtypes  # noqa: E402
import concourse.bass as bass  # noqa: E402
import concourse.mybir as mybir  # noqa: E402
import concourse.tile as tile  # noqa: E402
from concourse.tile import TileContext  # noqa: E402
from concourse.masks import make_identity  # noqa: E402
from concourse.vector_clock import ScopedClock, VectorClock  # noqa: E402

F32 = mybir.dt.float32
BF16 = mybir.dt.bfloat16
I32 = mybir.dt.int32
U32 = mybir.dt.uint32
BF16_NP = ml_dtypes.bfloat16

N_CORES = 8
L = 2048
E = 1024
H = 16
D = 64
HPC = H // N_CORES          # heads per core = 2
JC = L // N_CORES           # w_gumbel contraction chunk = 256
NCH = L // 128              # 16 m-chunks
SCALE = D ** -0.5           # 0.125
AF = mybir.ActivationFunctionType
ALU = mybir.AluOpType

WARMS = 24                  # PE keep-warm transposes during the collective


# ---------------------------------------------------------------------------
# Workarounds for this toolchain's walrus: it rejects instructions carrying
# more than ~2 semaphore waits, including the Tile tail drain.
# ---------------------------------------------------------------------------

def _patched_drain_and_barrier(self, tick_clock, wait_clock):
    gc = tick_clock.global_clock
    n = len(gc)
    for i in range(n):
        t = gc[i]
        if t > 0:
            vec = [0] * n
            vec[i] = t
            nop = self.nc.sync.nop()
            wait_clock.add_sem_waits(nop.ins, ScopedClock({None: VectorClock(vec)}))
    self.nc.sync.drain()  # waits already handled by the NOP cascade above
    self.nc.all_engine_barrier()
    assert self.sems is not None
    popped = self.nc._tile_sem_poison_stack.pop()
    assert popped is self._sem_poison
    self.nc.clear_and_free_semaphores(list(self.sems.allocated().values()))
    self.nc.all_engine_barrier()


tile.TileContext._drain_and_barrier = _patched_drain_and_barrier


def _split_excess_waits(nc, max_waits=1):
    nsplit = 0
    for fn in nc.m.functions:
        for blk in fn.blocks:
            insts = list(blk.instructions)
            new = []
            for ins in insts:
                si = ins.sync_info
                if si is not None and len(si.on_wait) > max_waits:
                    waits = list(si.on_wait)
                    keep = waits[-max_waits:]
                    for k, w in enumerate(waits[:-max_waits]):
                        nop = mybir.InstNoOp(name=f"{ins.name}-wsplit{k}")
                        nop.engine = ins.engine
                        nop.sync_info = mybir.SyncInfo(on_wait=[w], on_update=[])
                        new.append(nop)
                        nsplit += 1
                    si.on_wait = keep
                new.append(ins)
            blk.instructions = new
    return nsplit


# ---------------------------------------------------------------------------
# Device program
# ---------------------------------------------------------------------------

_CACHE = {}

_MASK2 = np.zeros((HPC, 128), np.float32)
for _h in range(HPC):
    _MASK2[_h, _h * D:(_h + 1) * D] = 1.0


def _build_program():
    nc = bass.Bass("TRN2", num_devices=N_CORES)

    qchunk = nc.dram_tensor("qchunk", [JC, E], F32, kind="ExternalInput")
    wt = nc.dram_tensor("wt", [JC, L], BF16, kind="ExternalInput")
    kht = nc.dram_tensor("kht", [HPC * D, L], BF16, kind="ExternalInput")
    vt1 = nc.dram_tensor("vt1", [L, 129], BF16, kind="ExternalInput")
    ufull = nc.dram_tensor("ufull", [H, L], F32, kind="ExternalInput")
    b8 = nc.dram_tensor("b8", [H, L], F32, kind="ExternalInput")
    qfull = nc.dram_tensor("qfull", [L * H, D], F32, kind="ExternalInput")
    hoff = nc.dram_tensor("hoff", [HPC, 1], I32, kind="ExternalInput")
    maskin = nc.dram_tensor("maskin", [HPC, 128], F32, kind="ExternalInput")
    outd = nc.dram_tensor("out", [L, HPC * D], F32, kind="ExternalOutput")

    lpart = nc.dram_tensor("lpart", [H, L], F32)
    lrs = nc.dram_tensor("lrs", [HPC, L], F32)

    with TileContext(nc) as tc:
        # PSUM: 8 banks total. mm:2 col:1 bro:1 warm:1 pq:1 sc:1 att:1 = 8
        with tc.tile_pool(name="big", bufs=1) as big, \
             tc.tile_pool(name="work", bufs=1) as work, \
             tc.tile_pool(name="ps_mm", bufs=2, space="PSUM") as ps_mm, \
             tc.tile_pool(name="ps_col", bufs=1, space="PSUM") as ps_col, \
             tc.tile_pool(name="ps_bro", bufs=1, space="PSUM") as ps_bro, \
             tc.tile_pool(name="ps_warm", bufs=1, space="PSUM") as ps_warm, \
             tc.tile_pool(name="ps_pq", bufs=1, space="PSUM") as ps_pq, \
             tc.tile_pool(name="ps_sc", bufs=1, space="PSUM") as ps_sc, \
             tc.tile_pool(name="ps_att", bufs=1, space="PSUM") as ps_att:

            ident = work.tile([128, 128], F32)
            make_identity(nc, ident)

            # ---- input loads -----------------------------------------------
            qt = big.tile([128, 2 * E], F32, tag="qt")
            nc.sync.dma_start(
                out=qt[:].rearrange("p (s e) -> p s e", e=E),
                in_=qchunk.rearrange("(s p) e -> p s e", p=128),
            )
            wtv = big.tile([128, 2 * L], BF16, tag="wtv")
            nc.sync.dma_start(
                out=wtv[:].rearrange("p (s i) -> p s i", i=L),
                in_=wt.rearrange("(s p) i -> p s i", p=128),
            )
            ut = big.tile([H, L], F32, tag="ut")
            nc.sync.dma_start(out=ut[:], in_=ufull[:])
            b8t = big.tile([H, L], F32, tag="b8t")
            nc.sync.dma_start(out=b8t[:], in_=b8[:])
            kt = big.tile([128, L], BF16, tag="kt")
            nc.sync.dma_start(out=kt[:], in_=kht[:])
            vt = big.tile([128, NCH * 129], BF16, tag="vt")
            nc.sync.dma_start(
                out=vt[:].rearrange("p (r c) -> p r c", c=129),
                in_=vt1.rearrange("(r p) c -> p r c", p=128),
            )
            hof = work.tile([HPC, 1], I32, tag="hof")
            nc.sync.dma_start(out=hof[:], in_=hoff[:])
            mask2 = work.tile([HPC, 128], F32, tag="mask2")
            nc.sync.dma_start(out=mask2[:], in_=maskin[:])

            # ---- q_mean^T (bf16 lhsT) --------------------------------------
            qm = work.tile([128, 2 * H], F32, tag="qm")
            for s in range(2):
                nc.vector.reduce_sum(
                    qm[:, s * H:(s + 1) * H],
                    qt[:, s * E:(s + 1) * E].rearrange("p (h d) -> p h d", d=D),
                    axis=mybir.AxisListType.X,
                )
            qmb = work.tile([128, 2 * H], BF16, tag="qmb")
            nc.vector.tensor_scalar_mul(qmb[:], qm[:], 1.0 / D)

            # ---- (b + gumbel)/8 on [16, L] (off critical path) -------------
            s1 = big.tile([H, L], F32, tag="s1")
            nc.scalar.activation(s1[:], ut[:], AF.Ln)
            s2 = big.tile([H, L], F32, tag="s2")
            nc.scalar.activation(s2[:], s1[:], AF.Ln, scale=-1.0)
            s28 = big.tile([H, L], F32, tag="s28")
            nc.vector.tensor_scalar_mul(s28[:], s2[:], -0.125)
            bs28 = big.tile([H, L], F32, tag="bs28")
            nc.vector.tensor_tensor(out=bs28[:], in0=s28[:], in1=b8t[:], op=ALU.add)

            # ---- partial logits + bs2/8, then ReduceScatter ----------------
            lp = big.tile([H, L], F32, tag="lp")
            for n in range(4):
                pl = ps_mm.tile([H, 512], F32, tag="mm")
                for s in range(2):
                    nc.tensor.matmul(
                        out=pl[:],
                        lhsT=qmb[:, s * H:(s + 1) * H],
                        rhs=wtv[:, s * L + n * 512: s * L + (n + 1) * 512],
                        start=(s == 0), stop=(s == 1),
                    )
                nc.vector.tensor_tensor(
                    out=lp[:, n * 512:(n + 1) * 512], in0=pl[:],
                    in1=bs28[:, n * 512:(n + 1) * 512], op=ALU.add,
                )
            nc.sync.dma_start(out=lpart[:], in_=lp[:])
            nc.gpsimd.collective_compute(
                "ReduceScatter", ALU.add,
                replica_groups=[list(range(N_CORES))],
                ins=[lpart[:]], outs=[lrs[:]],
            )
            lr = work.tile([HPC, L], F32, tag="lr")
            nc.sync.dma_start(out=lr[:], in_=lrs[:])

            # ---- v column sums + vmean broadcast (overlaps the RS) ---------
            ones1 = work.tile([128, 1], BF16, tag="ones1")
            nc.vector.memset(ones1[:], 1.0)
            pcol = ps_col.tile([1, 129], F32, tag="col")
            for r in range(NCH):
                nc.tensor.matmul(
                    out=pcol[:], lhsT=ones1[:],
                    rhs=vt[:, r * 129:(r + 1) * 129],
                    start=(r == 0), stop=(r == NCH - 1),
                )
            vmeanb = work.tile([1, 128], BF16, tag="vmeanb")
            nc.vector.tensor_scalar_mul(vmeanb[:], pcol[:, 0:128], 1.0 / L)
            ones_r = work.tile([1, 128], BF16, tag="ones_r")
            nc.vector.memset(ones_r[:], 1.0)
            pbro = ps_bro.tile([128, 128], F32, tag="bro")
            nc.tensor.matmul(out=pbro[:], lhsT=ones_r[:], rhs=vmeanb[:],
                             start=True, stop=True)
            vmb = work.tile([128, 128], F32, tag="vmb")
            nc.scalar.copy(vmb[:], pbro[:])
            ones12 = work.tile([1, HPC], BF16, tag="ones12")
            nc.vector.memset(ones12[:], 1.0)
            pv2 = ps_bro.tile([HPC, 128], F32, tag="bro")
            nc.tensor.matmul(out=pv2[:], lhsT=ones12[:], rhs=vmeanb[:],
                             start=True, stop=True)
            vme2 = work.tile([HPC, 128], F32, tag="vme2")
            nc.vector.tensor_copy(vme2[:], pv2[:])

            for r in range(NCH):
                nc.sync.dma_start(out=outd[r * 128:(r + 1) * 128, :], in_=vmb[:])

            # ---- keep PE busy across the collective ------------------------
            for wrm in range(WARMS):
                pw = ps_warm.tile([128, 128], F32, tag="warm", name=f"warm{wrm}")
                nc.tensor.transpose(out=pw[:], in_=qt[:, 0:128], identity=ident[:])

            # ---- argmax per head (fp32: min top-2 gap ~0.011) --------------
            mx = work.tile([HPC, 8], F32, tag="mx")
            idx = work.tile([HPC, 8], U32, tag="idx")
            nc.vector.max_with_indices(mx[:], idx[:], lr[:])
            idx_i = work.tile([HPC, 1], I32, tag="idx_i")
            nc.vector.tensor_copy(idx_i[:], idx[:, 0:1])
            fi = work.tile([HPC, 1], I32, tag="fi")
            nc.vector.tensor_scalar(out=fi[:], in0=idx_i[:], scalar1=H,
                                    scalar2=None, op0=ALU.mult)
            nc.vector.tensor_tensor(out=fi[:], in0=fi[:], in1=hof[:], op=ALU.add)

            # ---- gather the two selected q rows, build stacked q^T ---------
            qsel = work.tile([HPC, D], F32, tag="qsel")
            nc.gpsimd.indirect_dma_start(
                out=qsel[:], out_offset=None,
                in_=qfull[:, :],
                in_offset=bass.IndirectOffsetOnAxis(ap=fi[:, 0:1], axis=0),
            )
            pq = ps_pq.tile([D, HPC], F32, tag="pq")
            nc.tensor.transpose(out=pq[:], in_=qsel[:],
                                identity=ident[0:HPC, 0:HPC])
            qs2 = work.tile([D, HPC], BF16, tag="qs2")
            nc.vector.tensor_copy(qs2[:], pq[:])
            qbd = work.tile([128, HPC], BF16, tag="qbd")
            nc.vector.memset(qbd[:], 0.0)
            nc.vector.tensor_copy(qbd[0:D, 0:1], qs2[:, 0:1])
            nc.sync.dma_start(out=qbd[D:128, 1:2], in_=qs2[:, 1:2])

            # ---- one attention row per head (m-partitioned scores) ---------
            psc = ps_sc.tile([128, 2 * NCH], F32, tag="sc")
            for r in range(NCH):
                nc.tensor.matmul(
                    out=psc[:, 2 * r:2 * r + 2],
                    lhsT=kt[:, r * 128:(r + 1) * 128],
                    rhs=qbd[:], start=True, stop=True,
                )
            escb = work.tile([128, 2 * NCH], BF16, tag="escb")
            nc.scalar.activation(escb[:], psc[:], AF.Exp, scale=SCALE)
            patt = ps_att.tile([HPC, 129], F32, tag="att")
            for r in range(NCH):
                nc.tensor.matmul(
                    out=patt[:], lhsT=escb[:, 2 * r:2 * r + 2],
                    rhs=vt[:, r * 129:(r + 1) * 129],
                    start=(r == 0), stop=(r == NCH - 1),
                )

            # ---- fix rows: vmean + mask*(attn - vmean), indirect write -----
            rsum = work.tile([HPC, 1], F32, tag="rsum")
            nc.vector.reciprocal(rsum[:], patt[:, 128:129])
            att = work.tile([HPC, 128], F32, tag="att")
            nc.vector.tensor_scalar_mul(att[:], patt[:, 0:128], rsum[:, 0:1])
            delta = work.tile([HPC, 128], F32, tag="delta")
            nc.vector.tensor_tensor(out=delta[:], in0=att[:], in1=vme2[:],
                                    op=ALU.subtract)
            nc.vector.tensor_tensor(out=delta[:], in0=delta[:], in1=mask2[:],
                                    op=ALU.mult)
            fix = work.tile([HPC, 128], F32, tag="fix")
            nc.vector.tensor_tensor(out=fix[:], in0=delta[:], in1=vme2[:],
                                    op=ALU.add)
            nc.gpsimd.indirect_dma_start(
                out=outd[:, :],
                out_offset=bass.IndirectOffsetOnAxis(ap=idx_i[:, 0:1], axis=0),
                in_=fix[:], in_offset=None,
            )

    _split_excess_waits(nc)
    return nc


def _make_in_maps(inputs):
    query = np.ascontiguousarray(inputs["query"], dtype=np.float32)
    key = np.ascontiguousarray(inputs["key"], dtype=np.float32)
    value = np.ascontiguousarray(inputs["value"], dtype=np.float32)
    w_gumbel = np.ascontiguousarray(inputs["w_gumbel"], dtype=np.float32)
    b_gumbel = np.ascontiguousarray(inputs["b_gumbel"], dtype=np.float32)
    gumbel_u = np.ascontiguousarray(inputs["gumbel_u"], dtype=np.float32)

    q2 = query.reshape(L, E)
    k2 = key.reshape(L, E)
    v2 = value.reshape(L, E)
    qfull = np.ascontiguousarray(query.reshape(L * H, D))
    ufull = np.ascontiguousarray(gumbel_u[0])
    b8 = np.ascontiguousarray(
        np.broadcast_to(b_gumbel[None, :] * 0.125, (H, L)).astype(np.float32)
    )

    in_maps = []
    for c in range(N_CORES):
        cols = slice(c * HPC * D, (c + 1) * HPC * D)
        vt1 = np.empty((L, 129), BF16_NP)
        vt1[:, 0:128] = v2[:, cols].astype(BF16_NP)
        vt1[:, 128] = BF16_NP(1.0)
        in_maps.append({
            "qchunk": np.ascontiguousarray(q2[c * JC:(c + 1) * JC, :]),
            "wt": np.ascontiguousarray(
                w_gumbel[:, c * JC:(c + 1) * JC].T).astype(BF16_NP),
            "kht": np.ascontiguousarray(k2[:, cols].T).astype(BF16_NP),
            "vt1": vt1,
            "ufull": ufull,
            "b8": b8,
            "qfull": qfull,
            "hoff": np.array([[c * HPC], [c * HPC + 1]], dtype=np.int32),
            "maskin": _MASK2,
        })
    return in_maps


def kernel(query, key, value, w_gumbel, b_gumbel, gumbel_u):
    from concourse.bass_utils import run_bass_kernel_spmd

    if "nc" not in _CACHE:
        _CACHE["nc"] = _build_program()
    nc = _CACHE["nc"]

    in_maps = _make_in_maps({
        "query": query, "key": key, "value": value,
        "w_gumbel": w_gumbel, "b_gumbel": b_gumbel, "gumbel_u": gumbel_u,
    })
    res = run_bass_kernel_spmd(nc, in_maps, core_ids=list(range(N_CORES)))
    out = np.concatenate([res.results[c]["out"] for c in range(N_CORES)], axis=1)
    return out.reshape(1, L, E)


if __name__ == "__main__":
    rng = np.random.default_rng(0)
    ins = {
        "query": rng.standard_normal((1, L, E)).astype(np.float32),
        "key": rng.standard_normal((1, L, E)).astype(np.float32),
        "value": rng.standard_normal((1, L, E)).astype(np.float32),
        "w_gumbel": (rng.standard_normal((L, L)) * 0.02).astype(np.float32),
        "b_gumbel": np.zeros(L, np.float32),
        "gumbel_u": rng.uniform(1e-6, 1 - 1e-6, (1, H, L)).astype(np.float32),
    }
    out = kernel(**ins)
    print("out", out.shape, out.dtype, np.abs(out).max())
